# revision 8
# baseline (speedup 1.0000x reference)
"""Contrastive loss (GRACE-style semi_loss pair) on 8 trn2 NeuronCores.

Math (reference):
    a = z1 / ||z1||_row ; b = z2 / ||z2||_row         (N=8192, D=512)
    refl    = exp(a @ a.T / tau) ; between = exp(a @ b.T / tau)
    l1_i = -log(between_ii / (refl.sum(1) + between.sum(1) - refl_ii))
    l2   = same with (z2, z1) swapped
    loss = mean(0.5 * (l1 + l2))

Identities:
  - between2 (for l2) = between.T -> its row sums are COLUMN sums of
    exp(a@b.T/tau).
  - exp(a@a.T) and exp(b@b.T) are symmetric -> their row sums are also
    column sums.  All three column-sum families ride ONE ReduceScatter
    with a [core: ab|aa|bb] interleaved layout; no ACT accumulators or
    DVE row-reduces needed for aa/bb.
  - refl_ii = exp(1/tau) exactly; dab_i = a_i . b_i from fp8 diag blocks.
  - row sumsq (for 1/norm) = diag of the raw z Gram matrix, computed with
    fp8 DoubleRow diag blocks from a casting-DMA fp8 copy of z -- lands
    directly in [128, blocks] layout for a cheap 2-step Newton rsqrt.

Implementation (v4): single pass over zT; fp8e4 DoubleRow matmuls
(K=256/instr, 2x bf16 rate); aa|bb share one 2-bank PSUM tile and a
single [128,1024] exp.  Column sums: exp(ab) via delayed DVE adds,
exp(aa)/exp(bb) via delayed PE ones-matmul folds (one chunk behind so
neither engine waits on ACT).  Prep is stage-pipelined 2-3 units ahead.
Sharding: data-parallel rows; pinned fp8 stationary for the core's 1024
rows, all 16 512-col chunks streamed as moving operands.
"""

import os

# small collectives: RDH has a ~60-120us latency floor here; Mesh is ~10us.
os.environ.setdefault("NEURON_RT_DBG_RDH_CC", "0")

import numpy as np
from contextlib import ExitStack

KDEBUG = bool(os.environ.get("KDEBUG"))

import concourse.bass as bass
import concourse.tile as tile
from concourse import bacc, mybir
from concourse.bass_utils import run_bass_kernel_spmd

N = 8192
D = 512
P = 128
NCORES = 8
LOCAL = N // NCORES            # 1024 rows per core
M_CH = LOCAL // P              # 8 local row blocks of 128
N_UNITS = 8                    # 1024-column units
N_CH = 16                      # 512-column chunks
KC = D // P                    # 4 contraction chunks of 128
TAU = 0.4
SC = 16.0                      # fp8 operand scale: a~N(0,1/512) -> sigma .71
ESC = 1.0 / (SC * SC * TAU)    # exp() scale folding fp8 scaling + 1/tau
ISC2 = 1.0 / (SC * SC)
EXPD = float(np.exp(1.0 / TAU))
Y0 = float(D) ** -0.5          # Newton rsqrt seed; sumsq ~ 512 +- 6%

FP32 = mybir.dt.float32
BF16 = mybir.dt.bfloat16
FP16 = mybir.dt.float16
FP8 = mybir.dt.float8e4
ALU = mybir.AluOpType
ACTF = mybir.ActivationFunctionType
DR = mybir.MatmulPerfMode.DoubleRow
X_AX = mybir.AxisListType.X


def _build():
    nc = bacc.Bacc("TRN2", debug=False, num_devices=NCORES)
    z1T = nc.dram_tensor("z1T", [D, N], FP32, kind="ExternalInput").ap()
    z2T = nc.dram_tensor("z2T", [D, N], FP32, kind="ExternalInput").ap()
    z1lT = nc.dram_tensor("z1lT", [D, LOCAL], FP32, kind="ExternalInput").ap()
    z2lT = nc.dram_tensor("z2lT", [D, LOCAL], FP32, kind="ExternalInput").ap()
    eye = nc.dram_tensor("eye", [P, P], FP16, kind="ExternalInput").ap()
    # per-core one-hot row mask: rowmask[p, s*M_CH+m] = 1 iff slot s == core id
    rowmask = nc.dram_tensor("rowmask", [P, N // P], FP32, kind="ExternalInput").ap()
    loss = nc.dram_tensor("loss", [1, 1], FP32, kind="ExternalOutput").ap()
    if KDEBUG:
        dbg = {
            nm: nc.dram_tensor(f"dbg_{nm}", [P, N // P], FP32, kind="ExternalOutput").ap()
            for nm in ("cs_ab", "cs_aa", "cs_bb", "dab", "rs_ab", "d1", "d2")
        }

    with tile.TileContext(nc) as tc, ExitStack() as ctx:
        big = ctx.enter_context(tc.tile_pool(name="big", bufs=1))
        zst = ctx.enter_context(tc.tile_pool(name="zst", bufs=3))
        zrt = ctx.enter_context(tc.tile_pool(name="zrt", bufs=2))
        atp = ctx.enter_context(tc.tile_pool(name="atp", bufs=3))
        small = ctx.enter_context(tc.tile_pool(name="small", bufs=1))
        scratch = ctx.enter_context(tc.tile_pool(name="scratch", bufs=2))
        exa_pool = ctx.enter_context(tc.tile_pool(name="exa_pool", bufs=18))
        exp_pool = ctx.enter_context(tc.tile_pool(name="exp_pool", bufs=18))
        pa2 = ctx.enter_context(tc.tile_pool(name="pa2", bufs=2, space="PSUM"))
        pab = ctx.enter_context(tc.tile_pool(name="pab", bufs=2, space="PSUM"))
        psm = ctx.enter_context(tc.tile_pool(name="psm", bufs=2, space="PSUM"))
        dram = ctx.enter_context(tc.tile_pool(name="dram", bufs=1, space="DRAM"))

        # ---- constants --------------------------------------------------
        ones_bf = small.tile([P, 1], BF16, tag="ones_bf", name="ones_bf")
        nc.vector.memset(ones_bf, 1.0)
        ones_f32 = small.tile([P, 1], FP32, tag="ones_f32", name="ones_f32")
        nc.vector.memset(ones_f32, 1.0)
        eye_sb = small.tile([P, P], FP16, tag="eye", name="eye_sb")
        nc.sync.dma_start(out=eye_sb, in_=eye)

        # ---- persistent -------------------------------------------------
        dab = small.tile([P, M_CH], FP32, tag="dab", name="dab")
        rsp_ab = [
            small.tile([P, N_CH], FP32, tag=f"rsp_ab{m}", name=f"rsp_ab{m}")
            for m in range(M_CH)
        ]
        dtrash = small.tile([P, P], BF16, tag="dtrash", name="dtrash")

        # single fused AllReduce buffer, bf16, regions over all N=8192 rows:
        #   0: colsum exp(ab)   1: colsum exp(aa)   2: colsum exp(bb)
        #   3: dab (own rows, mask-zeroed elsewhere)   4: rowsum exp(ab) (ditto)
        # After the AR every core holds all global sums and computes the full
        # scalar loss redundantly -> no second collective, no RS latency.
        ccd_in = dram.tile([5, N], BF16, tag="ccd_in", name="ccd_in")
        ccd_out = dram.tile(
            [5, N], BF16, tag="ccd_out", name="ccd_out", addr_space="Shared"
        )
        mask_sb = small.tile([P, N // P], FP32, tag="mask_sb", name="mask_sb")
        nc.sync.dma_start(out=mask_sb, in_=rowmask)

        # ---- unit prep (staged) ----------------------------------------
        # s1: casting loads (bf16 + raw fp8), Gram-diag sumsq
        # s2: Newton rsqrt, broadcast round-trip
        # s3: fp8 operand scaling
        class Prep:
            pass

        def prep_s1(src1, src2, name):
            p_ = Prep()
            p_.name = name
            p_.zs1 = zst.tile([P, KC, 1024], BF16, tag="zs1", name=f"zs1_{name}")
            nc.gpsimd.dma_start(
                out=p_.zs1, in_=src1.rearrange("(k p) j -> p k j", p=P)
            )
            p_.zr1 = zrt.tile([P, KC, 1024], FP8, tag="zr1", name=f"zr1_{name}")
            nc.gpsimd.dma_start(
                out=p_.zr1, in_=src1.rearrange("(k p) j -> p k j", p=P)
            )
            p_.zs2 = zst.tile([P, KC, 1024], BF16, tag="zs2", name=f"zs2_{name}")
            nc.gpsimd.dma_start(
                out=p_.zs2, in_=src2.rearrange("(k p) j -> p k j", p=P)
            )
            p_.zr2 = zrt.tile([P, KC, 1024], FP8, tag="zr2", name=f"zr2_{name}")
            nc.gpsimd.dma_start(
                out=p_.zr2, in_=src2.rearrange("(k p) j -> p k j", p=P)
            )
            # sumsq of each column = diag of raw Gram diag blocks
            p_.ss = scratch.tile([P, 16], FP32, tag="ss", name=f"ss_{name}", bufs=3)
            for t, zr in ((0, p_.zr1), (1, p_.zr2)):
                for b in range(M_CH):
                    dps = psm.tile(
                        [P, P], FP32, tag="ps_small", name=f"gd_{name}_{t}_{b}"
                    )
                    for kp in range(2):
                        ks = slice(2 * kp, 2 * kp + 2)
                        nc.tensor.matmul(
                            dps, zr[:, ks, P * b : P * (b + 1)],
                            zr[:, ks, P * b : P * (b + 1)],
                            start=(kp == 0), stop=(kp == 1), perf_mode=DR,
                        )
                    nc.vector.scalar_tensor_tensor(
                        out=dtrash, in0=dps, scalar=1.0, in1=eye_sb,
                        op0=ALU.mult, op1=ALU.mult,
                        accum_out=p_.ss[:, 8 * t + b : 8 * t + b + 1],
                    )
            return p_

        def prep_s2(p_):
            name = p_.name
            ss_t = p_.ss
            # 2-step Newton for SC/sqrt(ss), SC folded into the last op
            y1 = scratch.tile([P, 16], FP32, tag="nw_y", name=f"y1_{name}")
            nc.vector.tensor_scalar(
                out=y1, in0=ss_t, scalar1=-0.5 * Y0**3, scalar2=1.5 * Y0,
                op0=ALU.mult, op1=ALU.add,
            )
            t = scratch.tile([P, 16], FP32, tag="nw_t", name=f"t_{name}")
            nc.vector.tensor_mul(t, y1, y1)
            nc.vector.tensor_mul(t, t, y1)
            nc.vector.scalar_tensor_tensor(
                out=t, in0=t, scalar=0.5, in1=ss_t, op0=ALU.mult, op1=ALU.mult
            )
            y2 = scratch.tile([P, 16], FP32, tag="nw_y2", name=f"y2_{name}")
            nc.vector.scalar_tensor_tensor(
                out=y2, in0=y1, scalar=1.5, in1=t, op0=ALU.mult, op1=ALU.subtract
            )
            nc.vector.tensor_mul(t, y2, y2)
            nc.vector.tensor_mul(t, t, y2)
            nc.vector.scalar_tensor_tensor(
                out=t, in0=t, scalar=0.5 * SC, in1=ss_t, op0=ALU.mult,
                op1=ALU.mult,
            )
            rl = scratch.tile([P, 16], FP16, tag="nw_rl", name=f"rl_{name}")
            nc.vector.scalar_tensor_tensor(
                out=rl, in0=y2, scalar=1.5 * SC, in1=t, op0=ALU.mult,
                op1=ALU.subtract,
            )
            rl_d = dram.tile([1, 2048], FP16, tag=f"rld_{name}", name=f"rld_{name}")
            nc.sync.dma_start(
                out=rl_d.rearrange("o (t p) -> p (o t)", p=P), in_=rl
            )
            p_.rb1 = scratch.tile(
                [P, 1024], FP16, tag="rb1", name=f"rb1_{name}", bufs=3
            )
            nc.sync.dma_start(
                out=p_.rb1, in_=rl_d[:, 0:1024].to_broadcast([P, 1024])
            )
            p_.rb2 = scratch.tile(
                [P, 1024], FP16, tag="rb2", name=f"rb2_{name}", bufs=3
            )
            nc.sync.dma_start(
                out=p_.rb2, in_=rl_d[:, 1024:2048].to_broadcast([P, 1024])
            )

        def prep_s3(p_, pin=False):
            name = p_.name
            if pin:
                at1 = big.tile([P, KC, 1024], FP8, tag="ATL1", name="ATL1")
                at2 = big.tile([P, KC, 1024], FP8, tag="ATL2", name="ATL2")
            else:
                at1 = atp.tile([P, KC, 1024], FP8, tag="at1", name=f"at1_{name}")
                at2 = atp.tile([P, KC, 1024], FP8, tag="at2", name=f"at2_{name}")
            for k in range(KC):
                eng = nc.gpsimd if k < 2 else nc.vector
                eng.tensor_mul(at1[:, k, :], p_.zs1[:, k, :], p_.rb1)
                eng = nc.gpsimd if k >= 2 else nc.vector
                eng.tensor_mul(at2[:, k, :], p_.zs2[:, k, :], p_.rb2)
            return at1, at2

        # ---- main loop pieces ------------------------------------------
        exabs = {}    # chunk n -> [8 exab tiles]
        exaabbs = {}  # chunk n -> [8 exaabb tiles]

        def main_chunk(n, at1, at2, ATL1, ATL2):
            h = 512 * (n % 2)
            exabs[n] = []
            exaabbs[n] = []
            for m in range(M_CH):
                a2 = pa2.tile([P, 2 * D], FP32, tag="a2", name=f"a2_{n}_{m}")
                ab = pab.tile([P, D], FP32, tag="ab", name=f"ab_{n}_{m}")
                lo, hi = P * m, P * (m + 1)
                for kp in range(2):
                    ks = slice(2 * kp, 2 * kp + 2)
                    st, sp = kp == 0, kp == 1
                    nc.tensor.matmul(
                        a2[:, 0:D], ATL1[:, ks, lo:hi], at1[:, ks, h : h + 512],
                        start=st, stop=sp, perf_mode=DR,
                    )
                    nc.tensor.matmul(
                        ab, ATL1[:, ks, lo:hi], at2[:, ks, h : h + 512],
                        start=st, stop=sp, perf_mode=DR,
                    )
                for kp in range(2):
                    ks = slice(2 * kp, 2 * kp + 2)
                    nc.tensor.matmul(
                        a2[:, D : 2 * D], ATL2[:, ks, lo:hi],
                        at2[:, ks, h : h + 512],
                        start=(kp == 0), stop=(kp == 1), perf_mode=DR,
                    )
                exaabb = exa_pool.tile(
                    [P, 2 * D], BF16, tag="exaabb", name=f"exaabb_{n}_{m}"
                )
                nc.scalar.activation(out=exaabb, in_=a2, func=ACTF.Exp, scale=ESC)
                exab = exp_pool.tile([P, D], BF16, tag="exab", name=f"exab_{n}_{m}")
                nc.scalar.activation(
                    out=exab, in_=ab, func=ACTF.Exp, scale=ESC,
                    accum_out=rsp_ab[m][:, n : n + 1],
                )
                exabs[n].append(exab)
                exaabbs[n].append(exaabb)

        def fold_chunk(n):
            """Delayed column-sum folds of chunk n: exp(ab) via DVE adds,
            exp(aa)/exp(bb) via PE ones-matmul accumulation.  All three
            [1,512] results ship in ONE strided DMA to the bf16 cc buffer."""
            colacc = scratch.tile(
                [P, D], BF16, tag="colacc", name=f"colacc_{n}", bufs=2
            )
            for m in range(M_CH):
                if m == 0:
                    nc.vector.tensor_copy(colacc, exabs[n][m])
                else:
                    nc.vector.tensor_add(colacc, colacc, exabs[n][m])
            colp = psm.tile([1, D], FP32, tag="ps_small", name=f"colp_{n}")
            nc.tensor.matmul(colp, ones_bf, colacc, start=True, stop=True)
            csb = scratch.tile([1, D], BF16, tag="csb", name=f"csb_{n}", bufs=2)
            nc.vector.tensor_copy(csb, colp)
            nc.sync.dma_start(out=ccd_in[0, 512 * n : 512 * (n + 1)], in_=csb)
            for r, half in ((1, 0), (2, 1)):  # aa, bb
                colq = psm.tile([1, D], FP32, tag="ps_small", name=f"colq{r}_{n}")
                for m in range(M_CH):
                    nc.tensor.matmul(
                        colq, ones_bf, exaabbs[n][m][:, 512 * half : 512 * half + 512],
                        start=(m == 0), stop=(m == M_CH - 1),
                    )
                csq = scratch.tile(
                    [1, D], BF16, tag=f"csq{r}", name=f"csq{r}_{n}", bufs=2
                )
                nc.vector.tensor_copy(csq, colq)
                nc.sync.dma_start(out=ccd_in[r, 512 * n : 512 * (n + 1)], in_=csq)
            del exabs[n]
            del exaabbs[n]

        def unit_src(u):
            return (
                z1T[:, 1024 * u : 1024 * (u + 1)],
                z2T[:, 1024 * u : 1024 * (u + 1)],
            )

        # ---- schedule ---------------------------------------------------
        # 3-deep stage pipeline: iter u runs s1(u+3), s2(u+2), s3(u+1)
        p_loc = prep_s1(z1lT, z2lT, "loc")
        preps = {}
        preps[0] = prep_s1(*unit_src(0), "u0")
        prep_s2(p_loc)
        preps[1] = prep_s1(*unit_src(1), "u1")
        prep_s2(preps[0])
        ATL1, ATL2 = prep_s3(p_loc, pin=True)
        preps[2] = prep_s1(*unit_src(2), "u2")
        prep_s2(preps[1])
        AT1, AT2 = {}, {}
        AT1[0], AT2[0] = prep_s3(preps[0])

        # dab: diag of local a.b product
        for m in range(M_CH):
            dps = psm.tile([P, P], FP32, tag="ps_small", name=f"dps_{m}")
            for kp in range(2):
                nc.tensor.matmul(
                    dps,
                    ATL1[:, 2 * kp : 2 * kp + 2, P * m : P * (m + 1)],
                    ATL2[:, 2 * kp : 2 * kp + 2, P * m : P * (m + 1)],
                    start=(kp == 0), stop=(kp == 1), perf_mode=DR,
                )
            nc.vector.scalar_tensor_tensor(
                out=dtrash, in0=dps, scalar=ISC2, in1=eye_sb,
                op0=ALU.mult, op1=ALU.mult, accum_out=dab[:, m : m + 1],
            )

        # scatter dab into its global row slots (zeros elsewhere via mask)
        dabm = small.tile([P, N // P], BF16, tag="dabm", name="dabm")
        for s in range(NCORES):
            sl = slice(M_CH * s, M_CH * (s + 1))
            nc.vector.tensor_mul(dabm[:, sl], dab, mask_sb[:, sl])
        nc.sync.dma_start(
            out=ccd_in[3].rearrange("(j p) -> p j", p=P), in_=dabm
        )

        AT1[1], AT2[1] = prep_s3(preps[1])
        for u in range(N_UNITS):
            if u + 3 < N_UNITS:
                preps[u + 3] = prep_s1(*unit_src(u + 3), f"u{u+3}")
            main_chunk(2 * u, AT1[u], AT2[u], ATL1, ATL2)
            if u + 2 < N_UNITS:
                prep_s2(preps[u + 2])
            if u > 0:
                fold_chunk(2 * u - 1)
            main_chunk(2 * u + 1, AT1[u], AT2[u], ATL1, ATL2)
            if u + 2 < N_UNITS:
                AT1[u + 2], AT2[u + 2] = prep_s3(preps[u + 2])
            fold_chunk(2 * u)
        fold_chunk(N_CH - 1)

        # ---- tail -------------------------------------------------------
        # local exp(ab) row sums -> masked scatter into global slots
        rs_ab = small.tile([P, M_CH], FP32, tag="rs_ab", name="rs_ab")
        for m in range(M_CH):
            nc.vector.reduce_sum(
                out=rs_ab[:, m : m + 1], in_=rsp_ab[m], axis=X_AX
            )
        rsabm = small.tile([P, N // P], BF16, tag="rsabm", name="rsabm")
        for s in range(NCORES):
            sl = slice(M_CH * s, M_CH * (s + 1))
            nc.vector.tensor_mul(rsabm[:, sl], rs_ab, mask_sb[:, sl])
        nc.sync.dma_start(
            out=ccd_in[4].rearrange("(j p) -> p j", p=P), in_=rsabm
        )

        nc.gpsimd.collective_compute(
            "AllReduce",
            ALU.add,
            replica_groups=[list(range(NCORES))],
            ins=[ccd_in.opt()],
            outs=[ccd_out.opt()],
        )

        JW = N // P  # 64 row-blocks of 128 over the full batch
        gt = {}
        for r, nm in enumerate(("cs_ab", "cs_aa", "cs_bb", "dab", "rs_ab")):
            gt[nm] = scratch.tile([P, JW], BF16, tag=f"g_{nm}", name=f"g_{nm}")
            nc.sync.dma_start(
                out=gt[nm], in_=ccd_out[r].rearrange("(j p) -> p j", p=P)
            )

        denom1 = small.tile([P, JW], FP32, tag="denom1", name="denom1")
        nc.vector.scalar_tensor_tensor(
            out=denom1, in0=gt["cs_aa"], scalar=-EXPD, in1=gt["rs_ab"],
            op0=ALU.add, op1=ALU.add,
        )
        denom2 = small.tile([P, JW], FP32, tag="denom2", name="denom2")
        nc.vector.scalar_tensor_tensor(
            out=denom2, in0=gt["cs_bb"], scalar=-EXPD, in1=gt["cs_ab"],
            op0=ALU.add, op1=ALU.add,
        )

        if KDEBUG:
            dvals = {
                "cs_ab": gt["cs_ab"], "cs_aa": gt["cs_aa"],
                "cs_bb": gt["cs_bb"], "dab": gt["dab"],
                "rs_ab": gt["rs_ab"], "d1": denom1, "d2": denom2,
            }
            for nm, t_ in dvals.items():
                cv = scratch.tile([P, JW], FP32, tag=f"dbg_{nm}", name=f"dbgc_{nm}")
                nc.vector.tensor_copy(cv, t_)
                nc.sync.dma_start(out=dbg[nm], in_=cv)

        nc.scalar.activation(out=denom1, in_=denom1, func=ACTF.Ln)
        nc.scalar.activation(out=denom2, in_=denom2, func=ACTF.Ln)
        nc.vector.tensor_add(denom1, denom1, denom2)  # ld1 + ld2

        combo = scratch.tile([P, JW], FP32, tag="combo", name="combo")
        ppart = small.tile([P, 1], FP32, tag="ppart", name="ppart")
        nc.vector.scalar_tensor_tensor(
            out=combo, in0=gt["dab"], scalar=-2.0 / TAU, in1=denom1,
            op0=ALU.mult, op1=ALU.add, accum_out=ppart,
        )
        lps = psm.tile([1, 1], FP32, tag="ps_small", name="lps")
        nc.tensor.matmul(lps, ones_f32, ppart, start=True, stop=True)
        lsb = small.tile([1, 1], FP32, tag="lsb", name="lsb")
        nc.scalar.mul(lsb, lps, 0.5 / N)
        nc.scalar.dma_start(out=loss, in_=lsb)

    nc.compile()
    return nc


_NC_CACHE = None


def _get_nc():
    global _NC_CACHE
    if _NC_CACHE is None:
        _NC_CACHE = _build()
    return _NC_CACHE


def _in_maps(z1, z2):
    z1 = np.ascontiguousarray(np.asarray(z1), dtype=np.float32)
    z2 = np.ascontiguousarray(np.asarray(z2), dtype=np.float32)
    z1T = np.ascontiguousarray(z1.T)
    z2T = np.ascontiguousarray(z2.T)
    eye = np.eye(P, dtype=np.float16)
    maps = []
    for c in range(NCORES):
        sl = slice(LOCAL * c, LOCAL * (c + 1))
        rowmask = np.zeros((P, N // P), dtype=np.float32)
        rowmask[:, M_CH * c : M_CH * (c + 1)] = 1.0
        maps.append(
            {
                "z1T": z1T,
                "z2T": z2T,
                "z1lT": np.ascontiguousarray(z1T[:, sl]),
                "z2lT": np.ascontiguousarray(z2T[:, sl]),
                "eye": eye,
                "rowmask": rowmask,
            }
        )
    return maps


def kernel(z1, z2):
    nc = _get_nc()
    res = run_bass_kernel_spmd(nc, _in_maps(z1, z2), list(range(NCORES)))
    return np.asarray(res.results[0]["loss"], dtype=np.float32).reshape(())


def kernel_traced(z1, z2):
    """Same as kernel() but with NTFF profiling; returns (loss, exec_time_ns,
    trace_path)."""
    import concourse.bass_utils as bu

    bu.upload_artifacts = lambda tmpdir: "local://" + tmpdir  # no egress
    nc = _get_nc()
    res = run_bass_kernel_spmd(
        nc, _in_maps(z1, z2), list(range(NCORES)), trace=True
    )
    out = np.asarray(res.results[0]["loss"], dtype=np.float32).reshape(())
    trace_path = (
        res.instructions_and_trace[1] if res.instructions_and_trace else None
    )
    return out, res.exec_time_ns, trace_path



# revision 9
# speedup vs baseline: 1.1101x; 1.1101x over previous
"""Contrastive loss (GRACE-style semi_loss pair) on 8 trn2 NeuronCores.

Math (reference):
    a = z1 / ||z1||_row ; b = z2 / ||z2||_row         (N=8192, D=512)
    refl    = exp(a @ a.T / tau) ; between = exp(a @ b.T / tau)
    l1_i = -log(between_ii / (refl.sum(1) + between.sum(1) - refl_ii))
    l2   = same with (z2, z1) swapped
    loss = mean(0.5 * (l1 + l2))

Identities:
  - between2 (for l2) = between.T -> its row sums are COLUMN sums of
    exp(a@b.T/tau).
  - exp(a@a.T) and exp(b@b.T) are symmetric -> their row sums are also
    column sums.  All three column-sum families ride ONE ReduceScatter
    with a [core: ab|aa|bb] interleaved layout; no ACT accumulators or
    DVE row-reduces needed for aa/bb.
  - refl_ii = exp(1/tau) exactly; dab_i = a_i . b_i from fp8 diag blocks.
  - row sumsq (for 1/norm) = diag of the raw z Gram matrix, computed with
    fp8 DoubleRow diag blocks from a casting-DMA fp8 copy of z -- lands
    directly in [128, blocks] layout for a cheap 2-step Newton rsqrt.

Implementation (v4): single pass over zT; fp8e4 DoubleRow matmuls
(K=256/instr, 2x bf16 rate); aa|bb share one 2-bank PSUM tile and a
single [128,1024] exp.  Column sums: exp(ab) via delayed DVE adds,
exp(aa)/exp(bb) via delayed PE ones-matmul folds (one chunk behind so
neither engine waits on ACT).  Prep is stage-pipelined 2-3 units ahead.
Sharding: data-parallel rows; pinned fp8 stationary for the core's 1024
rows, all 16 512-col chunks streamed as moving operands.
"""

import os

# small collectives: RDH has a ~60-120us latency floor here; Mesh is ~10us.
os.environ.setdefault("NEURON_RT_DBG_RDH_CC", "0")

import numpy as np
from contextlib import ExitStack

KDEBUG = bool(os.environ.get("KDEBUG"))

import concourse.bass as bass
import concourse.tile as tile
from concourse import bacc, mybir
from concourse.bass_utils import run_bass_kernel_spmd

N = 8192
D = 512
P = 128
NCORES = 8
LOCAL = N // NCORES            # 1024 rows per core
M_CH = LOCAL // P              # 8 local row blocks of 128
N_UNITS = 8                    # 1024-column units
N_CH = 16                      # 512-column chunks
KC = D // P                    # 4 contraction chunks of 128
TAU = 0.4
SC = 16.0                      # fp8 operand scale: a~N(0,1/512) -> sigma .71
ESC = 1.0 / (SC * SC * TAU)    # exp() scale folding fp8 scaling + 1/tau
ISC2 = 1.0 / (SC * SC)
EXPD = float(np.exp(1.0 / TAU))
Y0 = float(D) ** -0.5          # Newton rsqrt seed; sumsq ~ 512 +- 6%

FP32 = mybir.dt.float32
BF16 = mybir.dt.bfloat16
FP16 = mybir.dt.float16
FP8 = mybir.dt.float8e4
ALU = mybir.AluOpType
ACTF = mybir.ActivationFunctionType
DR = mybir.MatmulPerfMode.DoubleRow
X_AX = mybir.AxisListType.X


def _build():
    nc = bacc.Bacc("TRN2", debug=False, num_devices=NCORES)
    z1T = nc.dram_tensor("z1T", [D, N], FP32, kind="ExternalInput").ap()
    z2T = nc.dram_tensor("z2T", [D, N], FP32, kind="ExternalInput").ap()
    z1lT = nc.dram_tensor("z1lT", [D, LOCAL], FP32, kind="ExternalInput").ap()
    z2lT = nc.dram_tensor("z2lT", [D, LOCAL], FP32, kind="ExternalInput").ap()
    eye = nc.dram_tensor("eye", [P, P], FP16, kind="ExternalInput").ap()
    # per-core one-hot row mask: rowmask[p, s*M_CH+m] = 1 iff slot s == core id
    rowmask = nc.dram_tensor("rowmask", [P, N // P], FP32, kind="ExternalInput").ap()
    loss = nc.dram_tensor("loss", [1, 1], FP32, kind="ExternalOutput").ap()
    if KDEBUG:
        dbg = {
            nm: nc.dram_tensor(f"dbg_{nm}", [P, N // P], FP32, kind="ExternalOutput").ap()
            for nm in ("cs_ab", "cs_aa", "cs_bb", "dab", "rs_ab", "d1", "d2")
        }

    with tile.TileContext(nc) as tc, ExitStack() as ctx:
        big = ctx.enter_context(tc.tile_pool(name="big", bufs=1))
        zst = ctx.enter_context(tc.tile_pool(name="zst", bufs=3))
        zrt = ctx.enter_context(tc.tile_pool(name="zrt", bufs=2))
        atp = ctx.enter_context(tc.tile_pool(name="atp", bufs=3))
        small = ctx.enter_context(tc.tile_pool(name="small", bufs=1))
        scratch = ctx.enter_context(tc.tile_pool(name="scratch", bufs=2))
        exa_pool = ctx.enter_context(tc.tile_pool(name="exa_pool", bufs=18))
        exp_pool = ctx.enter_context(tc.tile_pool(name="exp_pool", bufs=18))
        pa2 = ctx.enter_context(tc.tile_pool(name="pa2", bufs=2, space="PSUM"))
        pab = ctx.enter_context(tc.tile_pool(name="pab", bufs=2, space="PSUM"))
        psm = ctx.enter_context(tc.tile_pool(name="psm", bufs=2, space="PSUM"))
        dram = ctx.enter_context(tc.tile_pool(name="dram", bufs=1, space="DRAM"))

        # ---- constants --------------------------------------------------
        ones_bf = small.tile([P, 1], BF16, tag="ones_bf", name="ones_bf")
        nc.vector.memset(ones_bf, 1.0)
        ones_f32 = small.tile([P, 1], FP32, tag="ones_f32", name="ones_f32")
        nc.vector.memset(ones_f32, 1.0)
        eye_sb = small.tile([P, P], FP16, tag="eye", name="eye_sb")
        nc.sync.dma_start(out=eye_sb, in_=eye)

        # ---- persistent -------------------------------------------------
        dab = small.tile([P, M_CH], FP32, tag="dab", name="dab")
        rsp_ab = [
            small.tile([P, N_CH], FP32, tag=f"rsp_ab{m}", name=f"rsp_ab{m}")
            for m in range(M_CH)
        ]
        dtrash = small.tile([P, P], BF16, tag="dtrash", name="dtrash")

        # single fused AllReduce buffer, bf16, regions over all N=8192 rows:
        #   0: colsum exp(ab)   1: colsum exp(aa)   2: colsum exp(bb)
        #   3: dab (own rows, mask-zeroed elsewhere)   4: rowsum exp(ab) (ditto)
        # After the AR every core holds all global sums and computes the full
        # scalar loss redundantly -> no second collective, no RS latency.
        ccd_in = dram.tile([5, N], BF16, tag="ccd_in", name="ccd_in")
        ccd_out = dram.tile(
            [5, N], BF16, tag="ccd_out", name="ccd_out", addr_space="Shared"
        )
        mask_sb = small.tile([P, N // P], FP32, tag="mask_sb", name="mask_sb")
        nc.sync.dma_start(out=mask_sb, in_=rowmask)

        # ---- unit prep (staged) ----------------------------------------
        # s1: casting loads (bf16 + raw fp8), Gram-diag sumsq
        # s2: Newton rsqrt, broadcast round-trip
        # s3: fp8 operand scaling
        class Prep:
            pass

        def prep_s1(src1, src2, name):
            p_ = Prep()
            p_.name = name
            p_.zs1 = zst.tile([P, KC, 1024], BF16, tag="zs1", name=f"zs1_{name}")
            nc.gpsimd.dma_start(
                out=p_.zs1, in_=src1.rearrange("(k p) j -> p k j", p=P)
            )
            p_.zr1 = zrt.tile([P, KC, 1024], FP8, tag="zr1", name=f"zr1_{name}")
            nc.gpsimd.dma_start(
                out=p_.zr1, in_=src1.rearrange("(k p) j -> p k j", p=P)
            )
            p_.zs2 = zst.tile([P, KC, 1024], BF16, tag="zs2", name=f"zs2_{name}")
            nc.gpsimd.dma_start(
                out=p_.zs2, in_=src2.rearrange("(k p) j -> p k j", p=P)
            )
            p_.zr2 = zrt.tile([P, KC, 1024], FP8, tag="zr2", name=f"zr2_{name}")
            nc.gpsimd.dma_start(
                out=p_.zr2, in_=src2.rearrange("(k p) j -> p k j", p=P)
            )
            # sumsq of each column = diag of raw Gram diag blocks
            p_.ss = scratch.tile([P, 16], FP32, tag="ss", name=f"ss_{name}", bufs=3)
            for t, zr in ((0, p_.zr1), (1, p_.zr2)):
                for b in range(M_CH):
                    dps = psm.tile(
                        [P, P], FP32, tag="ps_small", name=f"gd_{name}_{t}_{b}"
                    )
                    for kp in range(2):
                        ks = slice(2 * kp, 2 * kp + 2)
                        nc.tensor.matmul(
                            dps, zr[:, ks, P * b : P * (b + 1)],
                            zr[:, ks, P * b : P * (b + 1)],
                            start=(kp == 0), stop=(kp == 1), perf_mode=DR,
                        )
                    nc.vector.scalar_tensor_tensor(
                        out=dtrash, in0=dps, scalar=1.0, in1=eye_sb,
                        op0=ALU.mult, op1=ALU.mult,
                        accum_out=p_.ss[:, 8 * t + b : 8 * t + b + 1],
                    )
            return p_

        def prep_s2(p_):
            name = p_.name
            ss_t = p_.ss
            # 2-step Newton for SC/sqrt(ss), SC folded into the last op
            y1 = scratch.tile([P, 16], FP32, tag="nw_y", name=f"y1_{name}")
            nc.vector.tensor_scalar(
                out=y1, in0=ss_t, scalar1=-0.5 * Y0**3, scalar2=1.5 * Y0,
                op0=ALU.mult, op1=ALU.add,
            )
            t = scratch.tile([P, 16], FP32, tag="nw_t", name=f"t_{name}")
            nc.vector.tensor_mul(t, y1, y1)
            nc.vector.tensor_mul(t, t, y1)
            nc.vector.scalar_tensor_tensor(
                out=t, in0=t, scalar=0.5, in1=ss_t, op0=ALU.mult, op1=ALU.mult
            )
            y2 = scratch.tile([P, 16], FP32, tag="nw_y2", name=f"y2_{name}")
            nc.vector.scalar_tensor_tensor(
                out=y2, in0=y1, scalar=1.5, in1=t, op0=ALU.mult, op1=ALU.subtract
            )
            nc.vector.tensor_mul(t, y2, y2)
            nc.vector.tensor_mul(t, t, y2)
            nc.vector.scalar_tensor_tensor(
                out=t, in0=t, scalar=0.5 * SC, in1=ss_t, op0=ALU.mult,
                op1=ALU.mult,
            )
            rl = scratch.tile([P, 16], FP16, tag="nw_rl", name=f"rl_{name}")
            nc.vector.scalar_tensor_tensor(
                out=rl, in0=y2, scalar=1.5 * SC, in1=t, op0=ALU.mult,
                op1=ALU.subtract,
            )
            rl_d = dram.tile([1, 2048], FP16, tag=f"rld_{name}", name=f"rld_{name}")
            nc.sync.dma_start(
                out=rl_d.rearrange("o (t p) -> p (o t)", p=P), in_=rl
            )
            p_.rb1 = scratch.tile(
                [P, 1024], FP16, tag="rb1", name=f"rb1_{name}", bufs=3
            )
            nc.sync.dma_start(
                out=p_.rb1, in_=rl_d[:, 0:1024].to_broadcast([P, 1024])
            )
            p_.rb2 = scratch.tile(
                [P, 1024], FP16, tag="rb2", name=f"rb2_{name}", bufs=3
            )
            nc.sync.dma_start(
                out=p_.rb2, in_=rl_d[:, 1024:2048].to_broadcast([P, 1024])
            )

        def prep_s3(p_, pin=False):
            name = p_.name
            if pin:
                at1 = big.tile([P, KC, 1024], FP8, tag="ATL1", name="ATL1")
                at2 = big.tile([P, KC, 1024], FP8, tag="ATL2", name="ATL2")
            else:
                at1 = atp.tile([P, KC, 1024], FP8, tag="at1", name=f"at1_{name}")
                at2 = atp.tile([P, KC, 1024], FP8, tag="at2", name=f"at2_{name}")
            for k in range(KC):
                eng = nc.gpsimd if k < 2 else nc.vector
                eng.tensor_mul(at1[:, k, :], p_.zs1[:, k, :], p_.rb1)
                eng = nc.gpsimd if k >= 2 else nc.vector
                eng.tensor_mul(at2[:, k, :], p_.zs2[:, k, :], p_.rb2)
            return at1, at2

        # ---- main loop pieces ------------------------------------------
        exabs = {}    # chunk n -> [8 exab tiles]
        exaabbs = {}  # chunk n -> [8 exaabb tiles]

        def main_chunk(n, at1, at2, ATL1, ATL2):
            h = 512 * (n % 2)
            exabs[n] = []
            exaabbs[n] = []
            for m in range(M_CH):
                a2 = pa2.tile([P, 2 * D], FP32, tag="a2", name=f"a2_{n}_{m}")
                ab = pab.tile([P, D], FP32, tag="ab", name=f"ab_{n}_{m}")
                lo, hi = P * m, P * (m + 1)
                for kp in range(2):
                    ks = slice(2 * kp, 2 * kp + 2)
                    st, sp = kp == 0, kp == 1
                    nc.tensor.matmul(
                        a2[:, 0:D], ATL1[:, ks, lo:hi], at1[:, ks, h : h + 512],
                        start=st, stop=sp, perf_mode=DR,
                    )
                    nc.tensor.matmul(
                        ab, ATL1[:, ks, lo:hi], at2[:, ks, h : h + 512],
                        start=st, stop=sp, perf_mode=DR,
                    )
                for kp in range(2):
                    ks = slice(2 * kp, 2 * kp + 2)
                    nc.tensor.matmul(
                        a2[:, D : 2 * D], ATL2[:, ks, lo:hi],
                        at2[:, ks, h : h + 512],
                        start=(kp == 0), stop=(kp == 1), perf_mode=DR,
                    )
                exaabb = exa_pool.tile(
                    [P, 2 * D], BF16, tag="exaabb", name=f"exaabb_{n}_{m}"
                )
                nc.scalar.activation(out=exaabb, in_=a2, func=ACTF.Exp, scale=ESC)
                exab = exp_pool.tile([P, D], BF16, tag="exab", name=f"exab_{n}_{m}")
                nc.scalar.activation(
                    out=exab, in_=ab, func=ACTF.Exp, scale=ESC,
                    accum_out=rsp_ab[m][:, n : n + 1],
                )
                exabs[n].append(exab)
                exaabbs[n].append(exaabb)

        def fold_chunk(n):
            """Delayed column-sum folds of chunk n: exp(ab) via DVE adds,
            exp(aa)/exp(bb) via PE ones-matmul accumulation.  All three
            [1,512] results ship in ONE strided DMA to the bf16 cc buffer."""
            colacc = scratch.tile(
                [P, D], BF16, tag="colacc", name=f"colacc_{n}", bufs=2
            )
            for m in range(M_CH):
                if m == 0:
                    nc.vector.tensor_copy(colacc, exabs[n][m])
                else:
                    nc.vector.tensor_add(colacc, colacc, exabs[n][m])
            colp = psm.tile([1, D], FP32, tag="ps_small", name=f"colp_{n}")
            nc.tensor.matmul(colp, ones_bf, colacc, start=True, stop=True)
            csb = scratch.tile([1, D], BF16, tag="csb", name=f"csb_{n}", bufs=2)
            nc.vector.tensor_copy(csb, colp)
            nc.sync.dma_start(out=ccd_in[0, 512 * n : 512 * (n + 1)], in_=csb)
            for r, half in ((1, 0), (2, 1)):  # aa, bb
                colq = psm.tile([1, D], FP32, tag="ps_small", name=f"colq{r}_{n}")
                for m in range(M_CH):
                    nc.tensor.matmul(
                        colq, ones_bf, exaabbs[n][m][:, 512 * half : 512 * half + 512],
                        start=(m == 0), stop=(m == M_CH - 1),
                    )
                csq = scratch.tile(
                    [1, D], BF16, tag=f"csq{r}", name=f"csq{r}_{n}", bufs=2
                )
                nc.vector.tensor_copy(csq, colq)
                nc.sync.dma_start(out=ccd_in[r, 512 * n : 512 * (n + 1)], in_=csq)
            del exabs[n]
            del exaabbs[n]

        def unit_src(u):
            return (
                z1T[:, 1024 * u : 1024 * (u + 1)],
                z2T[:, 1024 * u : 1024 * (u + 1)],
            )

        # ---- schedule ---------------------------------------------------
        # 3-deep stage pipeline: iter u runs s1(u+3), s2(u+2), s3(u+1)
        p_loc = prep_s1(z1lT, z2lT, "loc")
        preps = {}
        preps[0] = prep_s1(*unit_src(0), "u0")
        prep_s2(p_loc)
        preps[1] = prep_s1(*unit_src(1), "u1")
        prep_s2(preps[0])
        ATL1, ATL2 = prep_s3(p_loc, pin=True)
        preps[2] = prep_s1(*unit_src(2), "u2")
        prep_s2(preps[1])
        AT1, AT2 = {}, {}
        AT1[0], AT2[0] = prep_s3(preps[0])

        # dab: diag of local a.b product
        for m in range(M_CH):
            dps = psm.tile([P, P], FP32, tag="ps_small", name=f"dps_{m}")
            for kp in range(2):
                nc.tensor.matmul(
                    dps,
                    ATL1[:, 2 * kp : 2 * kp + 2, P * m : P * (m + 1)],
                    ATL2[:, 2 * kp : 2 * kp + 2, P * m : P * (m + 1)],
                    start=(kp == 0), stop=(kp == 1), perf_mode=DR,
                )
            nc.vector.scalar_tensor_tensor(
                out=dtrash, in0=dps, scalar=ISC2, in1=eye_sb,
                op0=ALU.mult, op1=ALU.mult, accum_out=dab[:, m : m + 1],
            )

        # scatter dab into its global row slots (zeros elsewhere via mask)
        dabm = small.tile([P, N // P], BF16, tag="dabm", name="dabm")
        for s in range(NCORES):
            sl = slice(M_CH * s, M_CH * (s + 1))
            nc.vector.tensor_mul(dabm[:, sl], dab, mask_sb[:, sl])
        nc.sync.dma_start(
            out=ccd_in[3].rearrange("(j p) -> p j", p=P), in_=dabm
        )

        AT1[1], AT2[1] = prep_s3(preps[1])
        for u in range(N_UNITS):
            if u + 3 < N_UNITS:
                preps[u + 3] = prep_s1(*unit_src(u + 3), f"u{u+3}")
            main_chunk(2 * u, AT1[u], AT2[u], ATL1, ATL2)
            if u + 2 < N_UNITS:
                prep_s2(preps[u + 2])
            if u > 0:
                fold_chunk(2 * u - 1)
            main_chunk(2 * u + 1, AT1[u], AT2[u], ATL1, ATL2)
            if u + 2 < N_UNITS:
                AT1[u + 2], AT2[u + 2] = prep_s3(preps[u + 2])
            fold_chunk(2 * u)
        fold_chunk(N_CH - 1)

        # ---- tail -------------------------------------------------------
        # local exp(ab) row sums -> masked scatter into global slots
        rs_ab = small.tile([P, M_CH], FP32, tag="rs_ab", name="rs_ab")
        for m in range(M_CH):
            nc.vector.reduce_sum(
                out=rs_ab[:, m : m + 1], in_=rsp_ab[m], axis=X_AX
            )
        rsabm = small.tile([P, N // P], BF16, tag="rsabm", name="rsabm")
        for s in range(NCORES):
            sl = slice(M_CH * s, M_CH * (s + 1))
            nc.vector.tensor_mul(rsabm[:, sl], rs_ab, mask_sb[:, sl])
        nc.sync.dma_start(
            out=ccd_in[4].rearrange("(j p) -> p j", p=P), in_=rsabm
        )

        nc.gpsimd.collective_compute(
            "AllReduce",
            ALU.add,
            replica_groups=[list(range(NCORES))],
            ins=[ccd_in.opt()],
            outs=[ccd_out.opt()],
        )

        # contiguous readback: partition p holds rows 64p..64p+63.  The final
        # math is elementwise + full-sum, so row permutation is irrelevant —
        # it only has to be the SAME permutation for all five regions.
        JW = N // P  # 64 rows per partition
        gt = {}
        for r, nm in enumerate(("cs_ab", "cs_aa", "cs_bb", "dab", "rs_ab")):
            gt[nm] = scratch.tile([P, JW], BF16, tag=f"g_{nm}", name=f"g_{nm}")
            nc.sync.dma_start(
                out=gt[nm], in_=ccd_out[r].rearrange("(p j) -> p j", p=P)
            )

        denom1 = small.tile([P, JW], FP32, tag="denom1", name="denom1")
        nc.vector.scalar_tensor_tensor(
            out=denom1, in0=gt["cs_aa"], scalar=-EXPD, in1=gt["rs_ab"],
            op0=ALU.add, op1=ALU.add,
        )
        denom2 = small.tile([P, JW], FP32, tag="denom2", name="denom2")
        nc.vector.scalar_tensor_tensor(
            out=denom2, in0=gt["cs_bb"], scalar=-EXPD, in1=gt["cs_ab"],
            op0=ALU.add, op1=ALU.add,
        )

        if KDEBUG:
            dvals = {
                "cs_ab": gt["cs_ab"], "cs_aa": gt["cs_aa"],
                "cs_bb": gt["cs_bb"], "dab": gt["dab"],
                "rs_ab": gt["rs_ab"], "d1": denom1, "d2": denom2,
            }
            for nm, t_ in dvals.items():
                cv = scratch.tile([P, JW], FP32, tag=f"dbg_{nm}", name=f"dbgc_{nm}")
                nc.vector.tensor_copy(cv, t_)
                nc.sync.dma_start(out=dbg[nm], in_=cv)

        nc.scalar.activation(out=denom1, in_=denom1, func=ACTF.Ln)
        nc.scalar.activation(out=denom2, in_=denom2, func=ACTF.Ln)
        nc.vector.tensor_add(denom1, denom1, denom2)  # ld1 + ld2

        combo = scratch.tile([P, JW], FP32, tag="combo", name="combo")
        ppart = small.tile([P, 1], FP32, tag="ppart", name="ppart")
        nc.vector.scalar_tensor_tensor(
            out=combo, in0=gt["dab"], scalar=-2.0 / TAU, in1=denom1,
            op0=ALU.mult, op1=ALU.add, accum_out=ppart,
        )
        lps = psm.tile([1, 1], FP32, tag="ps_small", name="lps")
        nc.tensor.matmul(lps, ones_f32, ppart, start=True, stop=True)
        lsb = small.tile([1, 1], FP32, tag="lsb", name="lsb")
        nc.scalar.mul(lsb, lps, 0.5 / N)
        nc.scalar.dma_start(out=loss, in_=lsb)

    nc.compile()
    return nc


_NC_CACHE = None


def _get_nc():
    global _NC_CACHE
    if _NC_CACHE is None:
        _NC_CACHE = _build()
    return _NC_CACHE


def _in_maps(z1, z2):
    z1 = np.ascontiguousarray(np.asarray(z1), dtype=np.float32)
    z2 = np.ascontiguousarray(np.asarray(z2), dtype=np.float32)
    z1T = np.ascontiguousarray(z1.T)
    z2T = np.ascontiguousarray(z2.T)
    eye = np.eye(P, dtype=np.float16)
    maps = []
    for c in range(NCORES):
        sl = slice(LOCAL * c, LOCAL * (c + 1))
        rowmask = np.zeros((P, N // P), dtype=np.float32)
        rowmask[:, M_CH * c : M_CH * (c + 1)] = 1.0
        maps.append(
            {
                "z1T": z1T,
                "z2T": z2T,
                "z1lT": np.ascontiguousarray(z1T[:, sl]),
                "z2lT": np.ascontiguousarray(z2T[:, sl]),
                "eye": eye,
                "rowmask": rowmask,
            }
        )
    return maps


def kernel(z1, z2):
    nc = _get_nc()
    res = run_bass_kernel_spmd(nc, _in_maps(z1, z2), list(range(NCORES)))
    return np.asarray(res.results[0]["loss"], dtype=np.float32).reshape(())


def kernel_traced(z1, z2):
    """Same as kernel() but with NTFF profiling; returns (loss, exec_time_ns,
    trace_path)."""
    import concourse.bass_utils as bu

    bu.upload_artifacts = lambda tmpdir: "local://" + tmpdir  # no egress
    nc = _get_nc()
    res = run_bass_kernel_spmd(
        nc, _in_maps(z1, z2), list(range(NCORES)), trace=True
    )
    out = np.asarray(res.results[0]["loss"], dtype=np.float32).reshape(())
    trace_path = (
        res.instructions_and_trace[1] if res.instructions_and_trace else None
    )
    return out, res.exec_time_ns, trace_path



# revision 14
# speedup vs baseline: 1.1980x; 1.0792x over previous
"""Contrastive loss (GRACE-style semi_loss pair) on 8 trn2 NeuronCores.

Math (reference):
    a = z1 / ||z1||_row ; b = z2 / ||z2||_row         (N=8192, D=512)
    refl    = exp(a @ a.T / tau) ; between = exp(a @ b.T / tau)
    l1_i = -log(between_ii / (refl.sum(1) + between.sum(1) - refl_ii))
    l2   = same with (z2, z1) swapped
    loss = mean(0.5 * (l1 + l2))

Identities:
  - between2 (for l2) = between.T -> its row sums are COLUMN sums of
    exp(a@b.T/tau).
  - exp(a@a.T) and exp(b@b.T) are symmetric -> their row sums are also
    column sums.  All three column-sum families ride ONE ReduceScatter
    with a [core: ab|aa|bb] interleaved layout; no ACT accumulators or
    DVE row-reduces needed for aa/bb.
  - refl_ii = exp(1/tau) exactly; dab_i = a_i . b_i from fp8 diag blocks.
  - row sumsq (for 1/norm) = diag of the raw z Gram matrix, computed with
    fp8 DoubleRow diag blocks from a casting-DMA fp8 copy of z -- lands
    directly in [128, blocks] layout for a cheap 2-step Newton rsqrt.

Implementation (v4): single pass over zT; fp8e4 DoubleRow matmuls
(K=256/instr, 2x bf16 rate); aa|bb share one 2-bank PSUM tile and a
single [128,1024] exp.  Column sums: exp(ab) via delayed DVE adds,
exp(aa)/exp(bb) via delayed PE ones-matmul folds (one chunk behind so
neither engine waits on ACT).  Prep is stage-pipelined 2-3 units ahead.
Sharding: data-parallel rows; pinned fp8 stationary for the core's 1024
rows, all 16 512-col chunks streamed as moving operands.
"""

import os

# small collectives: RDH has a ~60-120us latency floor here; Mesh is ~10us.
os.environ.setdefault("NEURON_RT_DBG_RDH_CC", "0")

import numpy as np
from contextlib import ExitStack

KDEBUG = bool(os.environ.get("KDEBUG"))

import concourse.bass as bass
import concourse.tile as tile
from concourse import bacc, mybir
from concourse.bass_utils import run_bass_kernel_spmd

N = 8192
D = 512
P = 128
NCORES = 8
LOCAL = N // NCORES            # 1024 rows per core
M_CH = LOCAL // P              # 8 local row blocks of 128
N_UNITS = 8                    # 1024-column units
N_CH = 16                      # 512-column chunks
KC = D // P                    # 4 contraction chunks of 128
TAU = 0.4
SC = 16.0                      # fp8 operand scale: a~N(0,1/512) -> sigma .71
ESC = 1.0 / (SC * SC * TAU)    # exp() scale folding fp8 scaling + 1/tau
ISC2 = 1.0 / (SC * SC)
EXPD = float(np.exp(1.0 / TAU))
Y0 = float(D) ** -0.5          # Newton rsqrt seed; sumsq ~ 512 +- 6%

FP32 = mybir.dt.float32
BF16 = mybir.dt.bfloat16
FP16 = mybir.dt.float16
FP8 = mybir.dt.float8e4
ALU = mybir.AluOpType
ACTF = mybir.ActivationFunctionType
DR = mybir.MatmulPerfMode.DoubleRow
X_AX = mybir.AxisListType.X


def _build():
    nc = bacc.Bacc("TRN2", debug=False, num_devices=NCORES)
    # bf16 + raw-fp8 copies are prepared host-side so every device load is a
    # plain (non-cast) DMA: cast-DMA runs via SWDGE at ~60 GB/s aggregate,
    # plain HWDGE streams at ~350 GB/s.
    z1T = nc.dram_tensor("z1T", [D, N], BF16, kind="ExternalInput").ap()
    z2T = nc.dram_tensor("z2T", [D, N], BF16, kind="ExternalInput").ap()
    z1R = nc.dram_tensor("z1R", [D, N], FP8, kind="ExternalInput").ap()
    z2R = nc.dram_tensor("z2R", [D, N], FP8, kind="ExternalInput").ap()
    z1lT = nc.dram_tensor("z1lT", [D, LOCAL], BF16, kind="ExternalInput").ap()
    z2lT = nc.dram_tensor("z2lT", [D, LOCAL], BF16, kind="ExternalInput").ap()
    z1lR = nc.dram_tensor("z1lR", [D, LOCAL], FP8, kind="ExternalInput").ap()
    z2lR = nc.dram_tensor("z2lR", [D, LOCAL], FP8, kind="ExternalInput").ap()
    eye = nc.dram_tensor("eye", [P, P], FP16, kind="ExternalInput").ap()
    # per-core one-hot row mask: rowmask[p, s*M_CH+m] = 1 iff slot s == core id
    rowmask = nc.dram_tensor("rowmask", [P, N // P], FP32, kind="ExternalInput").ap()
    loss = nc.dram_tensor("loss", [1, 1], FP32, kind="ExternalOutput").ap()
    if KDEBUG:
        dbg = {
            nm: nc.dram_tensor(f"dbg_{nm}", [P, N // P], FP32, kind="ExternalOutput").ap()
            for nm in ("cs_ab", "cs_aa", "cs_bb", "dab", "rs_ab", "d1", "d2")
        }

    with tile.TileContext(nc) as tc, ExitStack() as ctx:
        big = ctx.enter_context(tc.tile_pool(name="big", bufs=1))
        zst = ctx.enter_context(tc.tile_pool(name="zst", bufs=3))
        zrt = ctx.enter_context(tc.tile_pool(name="zrt", bufs=2))
        atp = ctx.enter_context(tc.tile_pool(name="atp", bufs=3))
        small = ctx.enter_context(tc.tile_pool(name="small", bufs=1))
        scratch = ctx.enter_context(tc.tile_pool(name="scratch", bufs=2))
        exa_pool = ctx.enter_context(tc.tile_pool(name="exa_pool", bufs=18))
        exp_pool = ctx.enter_context(tc.tile_pool(name="exp_pool", bufs=18))
        pa2 = ctx.enter_context(tc.tile_pool(name="pa2", bufs=2, space="PSUM"))
        pab = ctx.enter_context(tc.tile_pool(name="pab", bufs=2, space="PSUM"))
        psm = ctx.enter_context(tc.tile_pool(name="psm", bufs=2, space="PSUM"))
        dram = ctx.enter_context(tc.tile_pool(name="dram", bufs=1, space="DRAM"))

        # ---- constants --------------------------------------------------
        ones_bf = small.tile([P, 1], BF16, tag="ones_bf", name="ones_bf")
        nc.vector.memset(ones_bf, 1.0)
        ones_f32 = small.tile([P, 1], FP32, tag="ones_f32", name="ones_f32")
        nc.vector.memset(ones_f32, 1.0)
        eye_sb = small.tile([P, P], FP16, tag="eye", name="eye_sb")
        nc.sync.dma_start(out=eye_sb, in_=eye)

        # ---- persistent -------------------------------------------------
        dab = small.tile([P, M_CH], FP32, tag="dab", name="dab")
        rsp_ab = [
            small.tile([P, N_CH], FP32, tag=f"rsp_ab{m}", name=f"rsp_ab{m}")
            for m in range(M_CH)
        ]
        dtrash = small.tile([P, P], BF16, tag="dtrash", name="dtrash")

        # single fused AllReduce buffer, bf16, regions over all N=8192 rows:
        #   0: colsum exp(ab)   1: colsum exp(aa)   2: colsum exp(bb)
        #   3: dab (own rows, mask-zeroed elsewhere)   4: rowsum exp(ab) (ditto)
        # After the AR every core holds all global sums and computes the full
        # scalar loss redundantly -> no second collective, no RS latency.
        ccd_in = dram.tile([5, N], BF16, tag="ccd_in", name="ccd_in")
        ccd_out = dram.tile(
            [5, N], BF16, tag="ccd_out", name="ccd_out", addr_space="Shared"
        )
        mask_sb = small.tile([P, N // P], FP32, tag="mask_sb", name="mask_sb")
        nc.sync.dma_start(out=mask_sb, in_=rowmask)

        # ---- unit prep (staged) ----------------------------------------
        # s1: casting loads (bf16 + raw fp8), Gram-diag sumsq
        # s2: Newton rsqrt, broadcast round-trip
        # s3: fp8 operand scaling
        class Prep:
            pass

        def prep_s1(src1, src2, raw1, raw2, name):
            p_ = Prep()
            p_.name = name
            p_.zs1 = zst.tile([P, KC, 1024], BF16, tag="zs1", name=f"zs1_{name}")
            nc.sync.dma_start(
                out=p_.zs1, in_=src1.rearrange("(k p) j -> p k j", p=P)
            )
            p_.zr1 = zrt.tile([P, KC, 1024], FP8, tag="zr1", name=f"zr1_{name}")
            nc.scalar.dma_start(
                out=p_.zr1, in_=raw1.rearrange("(k p) j -> p k j", p=P)
            )
            p_.zs2 = zst.tile([P, KC, 1024], BF16, tag="zs2", name=f"zs2_{name}")
            nc.scalar.dma_start(
                out=p_.zs2, in_=src2.rearrange("(k p) j -> p k j", p=P)
            )
            p_.zr2 = zrt.tile([P, KC, 1024], FP8, tag="zr2", name=f"zr2_{name}")
            nc.sync.dma_start(
                out=p_.zr2, in_=raw2.rearrange("(k p) j -> p k j", p=P)
            )
            # sumsq of each column = diag of raw Gram diag blocks
            p_.ss = scratch.tile([P, 16], FP32, tag="ss", name=f"ss_{name}", bufs=3)
            for t, zr in ((0, p_.zr1), (1, p_.zr2)):
                for b in range(M_CH):
                    dps = psm.tile(
                        [P, P], FP32, tag="ps_small", name=f"gd_{name}_{t}_{b}"
                    )
                    for kp in range(2):
                        ks = slice(2 * kp, 2 * kp + 2)
                        nc.tensor.matmul(
                            dps, zr[:, ks, P * b : P * (b + 1)],
                            zr[:, ks, P * b : P * (b + 1)],
                            start=(kp == 0), stop=(kp == 1), perf_mode=DR,
                        )
                    nc.vector.scalar_tensor_tensor(
                        out=dtrash, in0=dps, scalar=1.0, in1=eye_sb,
                        op0=ALU.mult, op1=ALU.mult,
                        accum_out=p_.ss[:, 8 * t + b : 8 * t + b + 1],
                    )
            return p_

        def prep_s2(p_):
            name = p_.name
            ss_t = p_.ss
            # 2-step Newton for SC/sqrt(ss), SC folded into the last op
            y1 = scratch.tile([P, 16], FP32, tag="nw_y", name=f"y1_{name}")
            nc.vector.tensor_scalar(
                out=y1, in0=ss_t, scalar1=-0.5 * Y0**3, scalar2=1.5 * Y0,
                op0=ALU.mult, op1=ALU.add,
            )
            t = scratch.tile([P, 16], FP32, tag="nw_t", name=f"t_{name}")
            nc.vector.tensor_mul(t, y1, y1)
            nc.vector.tensor_mul(t, t, y1)
            nc.vector.scalar_tensor_tensor(
                out=t, in0=t, scalar=0.5, in1=ss_t, op0=ALU.mult, op1=ALU.mult
            )
            y2 = scratch.tile([P, 16], FP32, tag="nw_y2", name=f"y2_{name}")
            nc.vector.scalar_tensor_tensor(
                out=y2, in0=y1, scalar=1.5, in1=t, op0=ALU.mult, op1=ALU.subtract
            )
            nc.vector.tensor_mul(t, y2, y2)
            nc.vector.tensor_mul(t, t, y2)
            nc.vector.scalar_tensor_tensor(
                out=t, in0=t, scalar=0.5 * SC, in1=ss_t, op0=ALU.mult,
                op1=ALU.mult,
            )
            rl = scratch.tile([P, 16], FP16, tag="nw_rl", name=f"rl_{name}")
            nc.vector.scalar_tensor_tensor(
                out=rl, in0=y2, scalar=1.5 * SC, in1=t, op0=ALU.mult,
                op1=ALU.subtract,
            )
            rl_d = dram.tile([1, 2048], FP16, tag=f"rld_{name}", name=f"rld_{name}")
            nc.sync.dma_start(
                out=rl_d.rearrange("o (t p) -> p (o t)", p=P), in_=rl
            )
            p_.rb1 = scratch.tile(
                [P, 1024], FP16, tag="rb1", name=f"rb1_{name}", bufs=3
            )
            nc.sync.dma_start(
                out=p_.rb1, in_=rl_d[:, 0:1024].to_broadcast([P, 1024])
            )
            p_.rb2 = scratch.tile(
                [P, 1024], FP16, tag="rb2", name=f"rb2_{name}", bufs=3
            )
            nc.sync.dma_start(
                out=p_.rb2, in_=rl_d[:, 1024:2048].to_broadcast([P, 1024])
            )

        def prep_s3(p_, pin=False):
            name = p_.name
            if pin:
                at1 = big.tile([P, KC, 1024], FP8, tag="ATL1", name="ATL1")
                at2 = big.tile([P, KC, 1024], FP8, tag="ATL2", name="ATL2")
            else:
                at1 = atp.tile([P, KC, 1024], FP8, tag="at1", name=f"at1_{name}")
                at2 = atp.tile([P, KC, 1024], FP8, tag="at2", name=f"at2_{name}")
            for k in range(KC):
                eng = nc.gpsimd if k < 2 else nc.vector
                eng.tensor_mul(at1[:, k, :], p_.zs1[:, k, :], p_.rb1)
                eng = nc.gpsimd if k >= 2 else nc.vector
                eng.tensor_mul(at2[:, k, :], p_.zs2[:, k, :], p_.rb2)
            return at1, at2

        # ---- main loop pieces ------------------------------------------
        exabs = {}    # chunk n -> [8 exab tiles]
        exaabbs = {}  # chunk n -> [8 exaabb tiles]

        def main_chunk(n, at1, at2, ATL1, ATL2):
            h = 512 * (n % 2)
            exabs[n] = []
            exaabbs[n] = []
            for m in range(M_CH):
                a2 = pa2.tile([P, 2 * D], FP32, tag="a2", name=f"a2_{n}_{m}")
                ab = pab.tile([P, D], FP32, tag="ab", name=f"ab_{n}_{m}")
                lo, hi = P * m, P * (m + 1)
                for kp in range(2):
                    ks = slice(2 * kp, 2 * kp + 2)
                    st, sp = kp == 0, kp == 1
                    nc.tensor.matmul(
                        a2[:, 0:D], ATL1[:, ks, lo:hi], at1[:, ks, h : h + 512],
                        start=st, stop=sp, perf_mode=DR,
                    )
                    nc.tensor.matmul(
                        ab, ATL1[:, ks, lo:hi], at2[:, ks, h : h + 512],
                        start=st, stop=sp, perf_mode=DR,
                    )
                for kp in range(2):
                    ks = slice(2 * kp, 2 * kp + 2)
                    nc.tensor.matmul(
                        a2[:, D : 2 * D], ATL2[:, ks, lo:hi],
                        at2[:, ks, h : h + 512],
                        start=(kp == 0), stop=(kp == 1), perf_mode=DR,
                    )
                exaabb = exa_pool.tile(
                    [P, 2 * D], BF16, tag="exaabb", name=f"exaabb_{n}_{m}"
                )
                nc.scalar.activation(out=exaabb, in_=a2, func=ACTF.Exp, scale=ESC)
                exab = exp_pool.tile([P, D], BF16, tag="exab", name=f"exab_{n}_{m}")
                nc.scalar.activation(
                    out=exab, in_=ab, func=ACTF.Exp, scale=ESC,
                    accum_out=rsp_ab[m][:, n : n + 1],
                )
                exabs[n].append(exab)
                exaabbs[n].append(exaabb)

        def fold_chunk(n):
            """Delayed column-sum folds of chunk n: exp(ab) via DVE adds,
            exp(aa)/exp(bb) via PE ones-matmul accumulation.  All three
            [1,512] results ship in ONE strided DMA to the bf16 cc buffer."""
            colacc = scratch.tile(
                [P, D], BF16, tag="colacc", name=f"colacc_{n}", bufs=2
            )
            for m in range(M_CH):
                if m == 0:
                    nc.vector.tensor_copy(colacc, exabs[n][m])
                else:
                    nc.vector.tensor_add(colacc, colacc, exabs[n][m])
            colp = psm.tile([1, D], FP32, tag="ps_small", name=f"colp_{n}")
            nc.tensor.matmul(colp, ones_bf, colacc, start=True, stop=True)
            csb = scratch.tile([1, D], BF16, tag="csb", name=f"csb_{n}", bufs=2)
            nc.vector.tensor_copy(csb, colp)
            nc.sync.dma_start(out=ccd_in[0, 512 * n : 512 * (n + 1)], in_=csb)
            for r, half in ((1, 0), (2, 1)):  # aa, bb
                colq = psm.tile([1, D], FP32, tag="ps_small", name=f"colq{r}_{n}")
                for m in range(M_CH):
                    nc.tensor.matmul(
                        colq, ones_bf, exaabbs[n][m][:, 512 * half : 512 * half + 512],
                        start=(m == 0), stop=(m == M_CH - 1),
                    )
                csq = scratch.tile(
                    [1, D], BF16, tag=f"csq{r}", name=f"csq{r}_{n}", bufs=2
                )
                nc.vector.tensor_copy(csq, colq)
                nc.sync.dma_start(out=ccd_in[r, 512 * n : 512 * (n + 1)], in_=csq)
            del exabs[n]
            del exaabbs[n]

        def unit_src(u):
            sl = slice(1024 * u, 1024 * (u + 1))
            return (z1T[:, sl], z2T[:, sl], z1R[:, sl], z2R[:, sl])

        # ---- schedule ---------------------------------------------------
        # 3-deep stage pipeline: iter u runs s1(u+3), s2(u+2), s3(u+1)
        p_loc = prep_s1(z1lT, z2lT, z1lR, z2lR, "loc")
        preps = {}
        preps[0] = prep_s1(*unit_src(0), "u0")
        prep_s2(p_loc)
        preps[1] = prep_s1(*unit_src(1), "u1")
        prep_s2(preps[0])
        ATL1, ATL2 = prep_s3(p_loc, pin=True)
        preps[2] = prep_s1(*unit_src(2), "u2")
        prep_s2(preps[1])
        AT1, AT2 = {}, {}
        AT1[0], AT2[0] = prep_s3(preps[0])

        # dab: diag of local a.b product
        for m in range(M_CH):
            dps = psm.tile([P, P], FP32, tag="ps_small", name=f"dps_{m}")
            for kp in range(2):
                nc.tensor.matmul(
                    dps,
                    ATL1[:, 2 * kp : 2 * kp + 2, P * m : P * (m + 1)],
                    ATL2[:, 2 * kp : 2 * kp + 2, P * m : P * (m + 1)],
                    start=(kp == 0), stop=(kp == 1), perf_mode=DR,
                )
            nc.vector.scalar_tensor_tensor(
                out=dtrash, in0=dps, scalar=ISC2, in1=eye_sb,
                op0=ALU.mult, op1=ALU.mult, accum_out=dab[:, m : m + 1],
            )

        # scatter dab into its global row slots (zeros elsewhere via mask)
        dabm = small.tile([P, N // P], BF16, tag="dabm", name="dabm")
        for s in range(NCORES):
            sl = slice(M_CH * s, M_CH * (s + 1))
            nc.vector.tensor_mul(dabm[:, sl], dab, mask_sb[:, sl])
        nc.sync.dma_start(
            out=ccd_in[3].rearrange("(j p) -> p j", p=P), in_=dabm
        )

        AT1[1], AT2[1] = prep_s3(preps[1])
        for u in range(N_UNITS):
            if u + 3 < N_UNITS:
                preps[u + 3] = prep_s1(*unit_src(u + 3), f"u{u+3}")
            main_chunk(2 * u, AT1[u], AT2[u], ATL1, ATL2)
            if u + 2 < N_UNITS:
                prep_s2(preps[u + 2])
            if u > 0:
                fold_chunk(2 * u - 1)
            main_chunk(2 * u + 1, AT1[u], AT2[u], ATL1, ATL2)
            if u + 2 < N_UNITS:
                AT1[u + 2], AT2[u + 2] = prep_s3(preps[u + 2])
            fold_chunk(2 * u)
        fold_chunk(N_CH - 1)

        # ---- tail -------------------------------------------------------
        # local exp(ab) row sums -> masked scatter into global slots
        rs_ab = small.tile([P, M_CH], FP32, tag="rs_ab", name="rs_ab")
        for m in range(M_CH):
            nc.vector.reduce_sum(
                out=rs_ab[:, m : m + 1], in_=rsp_ab[m], axis=X_AX
            )
        rsabm = small.tile([P, N // P], BF16, tag="rsabm", name="rsabm")
        for s in range(NCORES):
            sl = slice(M_CH * s, M_CH * (s + 1))
            nc.vector.tensor_mul(rsabm[:, sl], rs_ab, mask_sb[:, sl])
        nc.sync.dma_start(
            out=ccd_in[4].rearrange("(j p) -> p j", p=P), in_=rsabm
        )

        nc.gpsimd.collective_compute(
            "AllReduce",
            ALU.add,
            replica_groups=[list(range(NCORES))],
            ins=[ccd_in.opt()],
            outs=[ccd_out.opt()],
        )

        # contiguous readback: partition p holds rows 64p..64p+63.  The final
        # math is elementwise + full-sum, so row permutation is irrelevant —
        # it only has to be the SAME permutation for all five regions.
        JW = N // P  # 64 rows per partition
        gt = {}
        for r, nm in enumerate(("cs_ab", "cs_aa", "cs_bb", "dab", "rs_ab")):
            gt[nm] = scratch.tile([P, JW], BF16, tag=f"g_{nm}", name=f"g_{nm}")
            nc.sync.dma_start(
                out=gt[nm], in_=ccd_out[r].rearrange("(p j) -> p j", p=P)
            )

        denom1 = small.tile([P, JW], FP32, tag="denom1", name="denom1")
        nc.vector.scalar_tensor_tensor(
            out=denom1, in0=gt["cs_aa"], scalar=-EXPD, in1=gt["rs_ab"],
            op0=ALU.add, op1=ALU.add,
        )
        denom2 = small.tile([P, JW], FP32, tag="denom2", name="denom2")
        nc.vector.scalar_tensor_tensor(
            out=denom2, in0=gt["cs_bb"], scalar=-EXPD, in1=gt["cs_ab"],
            op0=ALU.add, op1=ALU.add,
        )

        if KDEBUG:
            dvals = {
                "cs_ab": gt["cs_ab"], "cs_aa": gt["cs_aa"],
                "cs_bb": gt["cs_bb"], "dab": gt["dab"],
                "rs_ab": gt["rs_ab"], "d1": denom1, "d2": denom2,
            }
            for nm, t_ in dvals.items():
                cv = scratch.tile([P, JW], FP32, tag=f"dbg_{nm}", name=f"dbgc_{nm}")
                nc.vector.tensor_copy(cv, t_)
                nc.sync.dma_start(out=dbg[nm], in_=cv)

        nc.scalar.activation(out=denom1, in_=denom1, func=ACTF.Ln)
        nc.scalar.activation(out=denom2, in_=denom2, func=ACTF.Ln)
        nc.vector.tensor_add(denom1, denom1, denom2)  # ld1 + ld2

        combo = scratch.tile([P, JW], FP32, tag="combo", name="combo")
        ppart = small.tile([P, 1], FP32, tag="ppart", name="ppart")
        nc.vector.scalar_tensor_tensor(
            out=combo, in0=gt["dab"], scalar=-2.0 / TAU, in1=denom1,
            op0=ALU.mult, op1=ALU.add, accum_out=ppart,
        )
        lps = psm.tile([1, 1], FP32, tag="ps_small", name="lps")
        nc.tensor.matmul(lps, ones_f32, ppart, start=True, stop=True)
        lsb = small.tile([1, 1], FP32, tag="lsb", name="lsb")
        nc.scalar.mul(lsb, lps, 0.5 / N)
        nc.scalar.dma_start(out=loss, in_=lsb)

    nc.compile()
    return nc


_NC_CACHE = None


def _get_nc():
    global _NC_CACHE
    if _NC_CACHE is None:
        _NC_CACHE = _build()
    return _NC_CACHE


def _in_maps(z1, z2):
    import ml_dtypes

    z1 = np.ascontiguousarray(np.asarray(z1), dtype=np.float32)
    z2 = np.ascontiguousarray(np.asarray(z2), dtype=np.float32)
    z1T = np.ascontiguousarray(z1.T)
    z2T = np.ascontiguousarray(z2.T)
    z1Tb = z1T.astype(ml_dtypes.bfloat16)
    z2Tb = z2T.astype(ml_dtypes.bfloat16)
    z1Tr = z1T.astype(ml_dtypes.float8_e4m3)
    z2Tr = z2T.astype(ml_dtypes.float8_e4m3)
    eye = np.eye(P, dtype=np.float16)
    maps = []
    for c in range(NCORES):
        sl = slice(LOCAL * c, LOCAL * (c + 1))
        rowmask = np.zeros((P, N // P), dtype=np.float32)
        rowmask[:, M_CH * c : M_CH * (c + 1)] = 1.0
        maps.append(
            {
                "z1T": z1Tb,
                "z2T": z2Tb,
                "z1R": z1Tr,
                "z2R": z2Tr,
                "z1lT": np.ascontiguousarray(z1Tb[:, sl]),
                "z2lT": np.ascontiguousarray(z2Tb[:, sl]),
                "z1lR": np.ascontiguousarray(z1Tr[:, sl]),
                "z2lR": np.ascontiguousarray(z2Tr[:, sl]),
                "eye": eye,
                "rowmask": rowmask,
            }
        )
    return maps


def kernel(z1, z2):
    nc = _get_nc()
    res = run_bass_kernel_spmd(nc, _in_maps(z1, z2), list(range(NCORES)))
    return np.asarray(res.results[0]["loss"], dtype=np.float32).reshape(())


def kernel_traced(z1, z2):
    """Same as kernel() but with NTFF profiling; returns (loss, exec_time_ns,
    trace_path)."""
    import concourse.bass_utils as bu

    bu.upload_artifacts = lambda tmpdir: "local://" + tmpdir  # no egress
    nc = _get_nc()
    res = run_bass_kernel_spmd(
        nc, _in_maps(z1, z2), list(range(NCORES)), trace=True
    )
    out = np.asarray(res.results[0]["loss"], dtype=np.float32).reshape(())
    trace_path = (
        res.instructions_and_trace[1] if res.instructions_and_trace else None
    )
    return out, res.exec_time_ns, trace_path

